# revision 7
# baseline (speedup 1.0000x reference)
"""GQA causal attention (B=1, S=2048, H=1024, 16 q-heads, 4 kv-heads, hd=64)
distributed over 8 TRN2 NeuronCores: tensor-parallel over query heads
(2 q-heads + their shared kv-head per core), x replicated. No collectives —
per-core output columns are concatenated on the host.

v2: bf16 matmul operands (fp32 matmul is 4 cyc/row LOW_HIGH on TRN2; bf16 is
1 cyc/row + FWL weight loads), fp32 PSUM accumulation and fp32 output path.
Seq-chunk streaming overlaps the x DMA with projections and attention
(causality makes chunk n's attention need only keys 0..n). Score matmuls are
K=64, row-packed in pairs at PE row groups 0/64 so two run concurrently.

Per-core layout (all SBUF-resident):
  qd0/qd1 [128, 2048] bf16 : per-head qT duplicated at partition bases 0 and 64
  ktd     [128, 2048] bf16 : kT duplicated at bases 0 and 64
  v1  [128, 16, 65]   bf16 : v tiles in [sk, hd] layout + ones column (denom)
  per head, 512-wide query chunk n, key-quad qi (causal: qi <= n):
     scoresT[j] = kT_ki.T @ q_chunk    [128, 4, 512] PSUM f32  (pairs packed)
     eq = exp(scoresT / 32) -> bf16    (one ACT op, N=2048)
     diagonal quad: zero invalid region in-place (GpSimd affine_select)
     o2 += [v_ki | 1].T @ eq[j]        [65, 512] PSUM f32 (row 64 = denominator)
  transpose o2 tiles back (PE, f32), normalize by row 64 (DVE), DMA out f32.
"""
from contextlib import ExitStack

import numpy as np
import ml_dtypes

import concourse.bass as bass
import concourse.tile as tile
from concourse import bacc, mybir
from concourse.bass_utils import run_bass_kernel_spmd

F32 = mybir.dt.float32
BF16 = mybir.dt.bfloat16
S = 2048
HID = 1024
NCORES = 8
SCALE = 1.0 / 32.0  # 1/sqrt(1024)
EXP = mybir.ActivationFunctionType.Exp


def _make_identity(nc, ap, size):
    nc.gpsimd.memset(ap, 0.0)
    nc.gpsimd.affine_select(
        out=ap,
        in_=ap,
        compare_op=mybir.AluOpType.not_equal,
        fill=1.0,
        base=0,
        pattern=[[-1, size]],
        channel_multiplier=1,
    )


def _build_kernel(ctx: ExitStack, tc: "tile.TileContext", out, xT, wq, wkv):
    nc = tc.nc

    const_pool = ctx.enter_context(tc.tile_pool(name="const", bufs=1))
    ident_bf = const_pool.tile([128, 128], BF16)

    persist = ctx.enter_context(tc.tile_pool(name="persist", bufs=1))
    qd0 = persist.tile([128, S], BF16)  # head0 qT at both partition bases
    qd1 = persist.tile([128, S], BF16)  # head1 qT at both partition bases
    ktd = persist.tile([128, S], BF16)  # kT at both partition bases
    vt = persist.tile([64, S], BF16)    # vT at base 0
    v1 = persist.tile([128, 16, 65], BF16)  # [v | 1] tiles, [sk, hd+1]
    wqsb = persist.tile([128, 8, 128], BF16)
    wkvsb = persist.tile([128, 8, 128], BF16)

    xpool = ctx.enter_context(tc.tile_pool(name="xin", bufs=2))
    xns = []
    for n in range(4):
        xn = xpool.tile([128, 4096], BF16, tag="xn", name=f"xn{n}", bufs=2)
        nc.sync.dma_start(xn[:, 0:2048], xT[n, :, 0:2048])
        nc.sync.dma_start(xn[:, 2048:4096], xT[n, :, 2048:4096])
        xns.append(xn)
    nc.sync.dma_start(wqsb[:], wq[:, :, :])
    nc.sync.dma_start(wkvsb[:], wkv[:, :, :])
    _make_identity(nc, ident_bf[:], 128)
    nc.vector.memset(v1[:, :, 64:65], 1.0)
    vtmp_pool = ctx.enter_context(tc.tile_pool(name="vtmp", bufs=2))
    ppsum = ctx.enter_context(tc.tile_pool(name="proj_psum", bufs=1, space="PSUM"))
    scp = ctx.enter_context(tc.tile_pool(name="sc_psum", bufs=1, space="PSUM"))
    o2p = ctx.enter_context(tc.tile_pool(name="o2_psum", bufs=1, space="PSUM"))
    trp = ctx.enter_context(tc.tile_pool(name="tr_psum", bufs=1, space="PSUM"))
    vtrp = trp  # share the single transpose-psum bank
    eqpool = ctx.enter_context(tc.tile_pool(name="eq", bufs=2))
    o2sbpool = ctx.enter_context(tc.tile_pool(name="o2sb", bufs=2))
    osbpool = ctx.enter_context(tc.tile_pool(name="osb", bufs=8))
    smallpool = ctx.enter_context(tc.tile_pool(name="small", bufs=4))

    for n in range(4):
        ns = slice(512 * n, 512 * (n + 1))
        # ---- projections for seq chunk n (accumulate over 8 hid chunks) ----
        xn = xns[n]
        pq = ppsum.tile([128, 512], F32, tag="pq")
        pkv = ppsum.tile([128, 512], F32, tag="pkv")
        for k in range(8):
            xs = xn[:, 512 * k:512 * (k + 1)]
            nc.tensor.matmul(
                pq[:], wqsb[:, k, :], xs, start=(k == 0), stop=(k == 7)
            )
            nc.tensor.matmul(
                pkv[:], wkvsb[:, k, :], xs, start=(k == 0), stop=(k == 7)
            )
        # cast copies to bf16 (DVE) + partition-base duplication (DMA)
        vtmp = vtmp_pool.tile([128, 512], BF16, tag="vtmp")
        nc.vector.tensor_copy(qd0[0:64, ns], pq[0:64, :])
        nc.vector.tensor_copy(qd1[64:128, ns], pq[64:128, :])
        nc.vector.tensor_copy(ktd[0:64, ns], pkv[0:64, :])
        nc.vector.tensor_copy(vtmp[64:128, :], pkv[64:128, :])
        nc.sync.dma_start(qd0[64:128, ns], qd0[0:64, ns])
        nc.sync.dma_start(qd1[0:64, ns], qd1[64:128, ns])
        nc.sync.dma_start(ktd[64:128, ns], ktd[0:64, ns])
        nc.sync.dma_start(vt[:, ns], vtmp[64:128, :])
        # [v | 1] tiles for this chunk's 4 key tiles
        for t in range(4 * n, 4 * n + 4):
            trv = vtrp.tile([128, 64], BF16, tag="trx")
            nc.tensor.transpose(
                trv[:], vt[:, 128 * t:128 * (t + 1)], ident_bf[0:64, 0:64]
            )
            nc.vector.tensor_copy(v1[:, t, 0:64], trv[:])

        # ---- attention for seq chunk n ----
        outs_n = [
            osbpool.tile([128, 128], F32, tag="osb", name=f"osb_n{n}_{t}")
            for t in range(4)
        ]
        nki = 4 * (n + 1)
        for h in range(2):
            qd = qd0 if h == 0 else qd1
            o2 = o2p.tile([65, 512], F32, tag="o2")
            for p in range(nki // 2):
                sq = scp.tile([128, 2, 512], F32, tag="sq", bufs=2)
                for j in range(2):
                    ki = 2 * p + j
                    b = 0 if (j % 2 == 0) else 64
                    nc.tensor.matmul(
                        sq[:, j, :],
                        ktd[b:b + 64, 128 * ki:128 * (ki + 1)],
                        qd[b:b + 64, ns],
                        start=True,
                        stop=True,
                    )
                eq = eqpool.tile([128, 2, 512], BF16, tag="eq", bufs=3)
                nc.scalar.activation(eq[:], sq[:], EXP, scale=SCALE)
                if 2 * p + 1 >= 4 * n:  # diagonal pair: zero invalid regions
                    for j in range(2):
                        ki = 2 * p + j
                        if ki >= 4 * n:
                            nc.gpsimd.affine_select(
                                out=eq[:, j, :],
                                in_=eq[:, j, :],
                                compare_op=mybir.AluOpType.is_ge,
                                fill=0.0,
                                base=512 * n - 128 * ki,
                                pattern=[[1, 512]],
                                channel_multiplier=-1,
                            )
                for j in range(2):
                    ki = 2 * p + j
                    nc.tensor.matmul(
                        o2[:],
                        v1[:, ki, :],
                        eq[:, j, :],
                        start=(ki == 0),
                        stop=(ki == nki - 1),
                    )
            o2sb = o2sbpool.tile([65, 512], BF16, tag="o2sb")
            nc.vector.tensor_copy(o2sb[:], o2[:])
            for t in range(4):
                tr = trp.tile([128, 65], BF16, tag="trx")
                nc.tensor.transpose(
                    tr[:], o2sb[:, 128 * t:128 * (t + 1)], ident_bf[0:65, 0:65]
                )
                rc = smallpool.tile([128, 1], F32, tag="rc")
                nc.vector.reciprocal(rc[:], tr[:, 64:65])
                nc.vector.tensor_scalar_mul(
                    outs_n[t][:, 64 * h:64 * (h + 1)], tr[:, 0:64], rc[:]
                )
        for t in range(4):
            nc.sync.dma_start(
                out[512 * n + 128 * t:512 * n + 128 * (t + 1), :], outs_n[t][:]
            )


def build_nc():
    nc = bacc.Bacc(
        "TRN2", target_bir_lowering=False, debug=False, num_devices=NCORES
    )
    xT = nc.dram_tensor("xT", [4, 128, 4096], BF16, kind="ExternalInput").ap()
    wq = nc.dram_tensor("wq", [128, 8, 128], BF16, kind="ExternalInput").ap()
    wkv = nc.dram_tensor("wkv", [128, 8, 128], BF16, kind="ExternalInput").ap()
    out = nc.dram_tensor("out", [S, 128], F32, kind="ExternalOutput").ap()
    with tile.TileContext(nc) as tc, ExitStack() as ctx:
        _build_kernel(ctx, tc, out, xT, wq, wkv)
    nc.compile()
    return nc


_NC_CACHE = None


def _get_nc():
    global _NC_CACHE
    if _NC_CACHE is None:
        _NC_CACHE = build_nc()
    return _NC_CACHE


def make_in_maps(x, Wq, Wk, Wv):
    x = np.asarray(x, dtype=np.float32)
    Wq = np.asarray(Wq, dtype=np.float32)
    Wk = np.asarray(Wk, dtype=np.float32)
    Wv = np.asarray(Wv, dtype=np.float32)
    bf = ml_dtypes.bfloat16
    xh = np.ascontiguousarray(
        x[0].reshape(4, 512, 8, 128).transpose(0, 3, 2, 1).reshape(4, 128, 4096)
    ).astype(bf)
    in_maps = []
    for d in range(NCORES):
        g = d // 2
        in_maps.append(
            {
                "xT": xh,
                "wq": np.ascontiguousarray(
                    Wq[128 * d:128 * (d + 1)].reshape(128, 8, 128).transpose(2, 1, 0)
                ).astype(bf),
                "wkv": np.ascontiguousarray(
                    np.concatenate(
                        [Wk[64 * g:64 * (g + 1)], Wv[64 * g:64 * (g + 1)]], axis=0
                    )
                    .reshape(128, 8, 128)
                    .transpose(2, 1, 0)
                ).astype(bf),
            }
        )
    return in_maps


def kernel(x, Wq, Wk, Wv):
    in_maps = make_in_maps(x, Wq, Wk, Wv)
    res = run_bass_kernel_spmd(_get_nc(), in_maps, core_ids=list(range(NCORES)))
    outs = [res.results[d]["out"] for d in range(NCORES)]
    return np.concatenate(outs, axis=1)[None, :, :]


# revision 8
# speedup vs baseline: 1.0170x; 1.0170x over previous
"""GQA causal attention (B=1, S=2048, H=1024, 16 q-heads, 4 kv-heads, hd=64)
distributed over 8 TRN2 NeuronCores: tensor-parallel over query heads
(2 q-heads + their shared kv-head per core), x replicated. No collectives —
per-core output columns are concatenated on the host.

v2: bf16 matmul operands (fp32 matmul is 4 cyc/row LOW_HIGH on TRN2; bf16 is
1 cyc/row + FWL weight loads), fp32 PSUM accumulation and fp32 output path.
Seq-chunk streaming overlaps the x DMA with projections and attention
(causality makes chunk n's attention need only keys 0..n). Score matmuls are
K=64, row-packed in pairs at PE row groups 0/64 so two run concurrently.

Per-core layout (all SBUF-resident):
  qd0/qd1 [128, 2048] bf16 : per-head qT duplicated at partition bases 0 and 64
  ktd     [128, 2048] bf16 : kT duplicated at bases 0 and 64
  v1  [128, 16, 65]   bf16 : v tiles in [sk, hd] layout + ones column (denom)
  per head, 512-wide query chunk n, key-quad qi (causal: qi <= n):
     scoresT[j] = kT_ki.T @ q_chunk    [128, 4, 512] PSUM f32  (pairs packed)
     eq = exp(scoresT / 32) -> bf16    (one ACT op, N=2048)
     diagonal quad: zero invalid region in-place (GpSimd affine_select)
     o2 += [v_ki | 1].T @ eq[j]        [65, 512] PSUM f32 (row 64 = denominator)
  transpose o2 tiles back (PE, f32), normalize by row 64 (DVE), DMA out f32.
"""
from contextlib import ExitStack

import numpy as np
import ml_dtypes

import concourse.bass as bass
import concourse.tile as tile
from concourse import bacc, mybir
from concourse.bass_utils import run_bass_kernel_spmd

F32 = mybir.dt.float32
BF16 = mybir.dt.bfloat16
S = 2048
HID = 1024
NCORES = 8
SCALE = 1.0 / 32.0  # 1/sqrt(1024)
EXP = mybir.ActivationFunctionType.Exp


def _make_identity(nc, ap, size):
    nc.gpsimd.memset(ap, 0.0)
    nc.gpsimd.affine_select(
        out=ap,
        in_=ap,
        compare_op=mybir.AluOpType.not_equal,
        fill=1.0,
        base=0,
        pattern=[[-1, size]],
        channel_multiplier=1,
    )


def _build_kernel(ctx: ExitStack, tc: "tile.TileContext", out, xT, wq, wkv):
    nc = tc.nc

    const_pool = ctx.enter_context(tc.tile_pool(name="const", bufs=1))
    ident_bf = const_pool.tile([128, 128], BF16)

    persist = ctx.enter_context(tc.tile_pool(name="persist", bufs=1))
    qd0 = persist.tile([128, S], BF16)  # head0 qT at both partition bases
    qd1 = persist.tile([128, S], BF16)  # head1 qT at both partition bases
    ktd = persist.tile([128, S], BF16)  # kT at both partition bases
    vt = persist.tile([64, S], BF16)    # vT at base 0
    v1 = persist.tile([128, 16, 65], BF16)  # [v | 1] tiles, [sk, hd+1]
    wqsb = persist.tile([128, 8, 128], BF16)
    wkvsb = persist.tile([128, 8, 128], BF16)

    nc.sync.dma_start(wqsb[:], wq[:, :, :])
    nc.sync.dma_start(wkvsb[:], wkv[:, :, :])
    xpool = ctx.enter_context(tc.tile_pool(name="xin", bufs=2))
    xns = []
    for n in range(4):
        xn = xpool.tile([128, 4096], BF16, tag="xn", name=f"xn{n}", bufs=2)
        nsplit = 4 if n == 0 else 2
        w = 4096 // nsplit
        for i in range(nsplit):
            nc.sync.dma_start(xn[:, w * i:w * (i + 1)], xT[n, :, w * i:w * (i + 1)])
        xns.append(xn)
    _make_identity(nc, ident_bf[:], 128)
    nc.vector.memset(v1[:, :, 64:65], 1.0)
    vtmp_pool = ctx.enter_context(tc.tile_pool(name="vtmp", bufs=2))
    ppsum = ctx.enter_context(tc.tile_pool(name="proj_psum", bufs=1, space="PSUM"))
    scp = ctx.enter_context(tc.tile_pool(name="sc_psum", bufs=1, space="PSUM"))
    o2p = ctx.enter_context(tc.tile_pool(name="o2_psum", bufs=1, space="PSUM"))
    trp = ctx.enter_context(tc.tile_pool(name="tr_psum", bufs=1, space="PSUM"))
    vtrp = trp  # share the single transpose-psum bank
    eqpool = ctx.enter_context(tc.tile_pool(name="eq", bufs=2))
    o2sbpool = ctx.enter_context(tc.tile_pool(name="o2sb", bufs=2))
    osbpool = ctx.enter_context(tc.tile_pool(name="osb", bufs=8))
    smallpool = ctx.enter_context(tc.tile_pool(name="small", bufs=4))

    for n in range(4):
        ns = slice(512 * n, 512 * (n + 1))
        # ---- projections for seq chunk n (accumulate over 8 hid chunks) ----
        xn = xns[n]
        pq = ppsum.tile([128, 512], F32, tag="pq")
        pkv = ppsum.tile([128, 512], F32, tag="pkv")
        for k in range(8):
            xs = xn[:, 512 * k:512 * (k + 1)]
            nc.tensor.matmul(
                pq[:], wqsb[:, k, :], xs, start=(k == 0), stop=(k == 7)
            )
            nc.tensor.matmul(
                pkv[:], wkvsb[:, k, :], xs, start=(k == 0), stop=(k == 7)
            )
        # cast copies to bf16 (DVE) + partition-base duplication (DMA)
        vtmp = vtmp_pool.tile([128, 512], BF16, tag="vtmp")
        nc.vector.tensor_copy(qd0[0:64, ns], pq[0:64, :])
        nc.vector.tensor_copy(qd1[64:128, ns], pq[64:128, :])
        nc.vector.tensor_copy(ktd[0:64, ns], pkv[0:64, :])
        nc.vector.tensor_copy(vtmp[64:128, :], pkv[64:128, :])
        nc.gpsimd.dma_start(qd0[64:128, ns], qd0[0:64, ns])
        nc.gpsimd.dma_start(qd1[0:64, ns], qd1[64:128, ns])
        nc.gpsimd.dma_start(ktd[64:128, ns], ktd[0:64, ns])
        nc.gpsimd.dma_start(vt[:, ns], vtmp[64:128, :])
        # [v | 1] tiles for this chunk's 4 key tiles
        for t in range(4 * n, 4 * n + 4):
            trv = vtrp.tile([128, 64], BF16, tag="trx")
            nc.tensor.transpose(
                trv[:], vt[:, 128 * t:128 * (t + 1)], ident_bf[0:64, 0:64]
            )
            nc.vector.tensor_copy(v1[:, t, 0:64], trv[:])

        # ---- attention for seq chunk n ----
        outs_n = [
            osbpool.tile([128, 128], F32, tag="osb", name=f"osb_n{n}_{t}")
            for t in range(4)
        ]
        nki = 4 * (n + 1)
        for h in range(2):
            qd = qd0 if h == 0 else qd1
            o2 = o2p.tile([65, 512], F32, tag="o2")
            for p in range(nki // 2):
                sq = scp.tile([128, 2, 512], F32, tag="sq", bufs=2)
                for j in range(2):
                    ki = 2 * p + j
                    b = 0 if (j % 2 == 0) else 64
                    nc.tensor.matmul(
                        sq[:, j, :],
                        ktd[b:b + 64, 128 * ki:128 * (ki + 1)],
                        qd[b:b + 64, ns],
                        start=True,
                        stop=True,
                    )
                eq = eqpool.tile([128, 2, 512], BF16, tag="eq", bufs=3)
                nc.scalar.activation(eq[:], sq[:], EXP, scale=SCALE)
                if 2 * p + 1 >= 4 * n:  # diagonal pair: zero invalid regions
                    for j in range(2):
                        ki = 2 * p + j
                        if ki >= 4 * n:
                            nc.gpsimd.affine_select(
                                out=eq[:, j, :],
                                in_=eq[:, j, :],
                                compare_op=mybir.AluOpType.is_ge,
                                fill=0.0,
                                base=512 * n - 128 * ki,
                                pattern=[[1, 512]],
                                channel_multiplier=-1,
                            )
                for j in range(2):
                    ki = 2 * p + j
                    nc.tensor.matmul(
                        o2[:],
                        v1[:, ki, :],
                        eq[:, j, :],
                        start=(ki == 0),
                        stop=(ki == nki - 1),
                    )
            o2sb = o2sbpool.tile([65, 512], BF16, tag="o2sb")
            nc.vector.tensor_copy(o2sb[:], o2[:])
            for t in range(4):
                tr = trp.tile([128, 65], BF16, tag="trx")
                nc.tensor.transpose(
                    tr[:], o2sb[:, 128 * t:128 * (t + 1)], ident_bf[0:65, 0:65]
                )
                rc = smallpool.tile([128, 1], F32, tag="rc")
                nc.vector.reciprocal(rc[:], tr[:, 64:65])
                nc.vector.tensor_scalar_mul(
                    outs_n[t][:, 64 * h:64 * (h + 1)], tr[:, 0:64], rc[:]
                )
        for t in range(4):
            nc.sync.dma_start(
                out[512 * n + 128 * t:512 * n + 128 * (t + 1), :], outs_n[t][:]
            )


def build_nc():
    nc = bacc.Bacc(
        "TRN2", target_bir_lowering=False, debug=False, num_devices=NCORES
    )
    xT = nc.dram_tensor("xT", [4, 128, 4096], BF16, kind="ExternalInput").ap()
    wq = nc.dram_tensor("wq", [128, 8, 128], BF16, kind="ExternalInput").ap()
    wkv = nc.dram_tensor("wkv", [128, 8, 128], BF16, kind="ExternalInput").ap()
    out = nc.dram_tensor("out", [S, 128], F32, kind="ExternalOutput").ap()
    with tile.TileContext(nc) as tc, ExitStack() as ctx:
        _build_kernel(ctx, tc, out, xT, wq, wkv)
    nc.compile()
    return nc


_NC_CACHE = None


def _get_nc():
    global _NC_CACHE
    if _NC_CACHE is None:
        _NC_CACHE = build_nc()
    return _NC_CACHE


def make_in_maps(x, Wq, Wk, Wv):
    x = np.asarray(x, dtype=np.float32)
    Wq = np.asarray(Wq, dtype=np.float32)
    Wk = np.asarray(Wk, dtype=np.float32)
    Wv = np.asarray(Wv, dtype=np.float32)
    bf = ml_dtypes.bfloat16
    xh = np.ascontiguousarray(
        x[0].reshape(4, 512, 8, 128).transpose(0, 3, 2, 1).reshape(4, 128, 4096)
    ).astype(bf)
    in_maps = []
    for d in range(NCORES):
        g = d // 2
        in_maps.append(
            {
                "xT": xh,
                "wq": np.ascontiguousarray(
                    Wq[128 * d:128 * (d + 1)].reshape(128, 8, 128).transpose(2, 1, 0)
                ).astype(bf),
                "wkv": np.ascontiguousarray(
                    np.concatenate(
                        [Wk[64 * g:64 * (g + 1)], Wv[64 * g:64 * (g + 1)]], axis=0
                    )
                    .reshape(128, 8, 128)
                    .transpose(2, 1, 0)
                ).astype(bf),
            }
        )
    return in_maps


def kernel(x, Wq, Wk, Wv):
    in_maps = make_in_maps(x, Wq, Wk, Wv)
    res = run_bass_kernel_spmd(_get_nc(), in_maps, core_ids=list(range(NCORES)))
    outs = [res.results[d]["out"] for d in range(NCORES)]
    return np.concatenate(outs, axis=1)[None, :, :]
